# revision 6
# baseline (speedup 1.0000x reference)
"""Trainium2 Bass kernel v2 for nn_FCPairedLayer (pairwise MLP edge scorer).

Math (B=2, N=1024, C=128, H1=128, H2=64):
    aTb1 = (x @ W1[:C] + b1).T per token; rT = (x @ W1[C:]).T per token
    H_i  = relu(rT[:, j] + aTb1[:, i])        [H1=128, w] per row i
    ph   = W2.T @ H (2 i's col-stacked on PE)  -> PSUM
    h2s  = relu(ph + b2)                       -> SBUF bf16
    y    = w3 . h2s (M=2 matmuls, 4 v-slots)   -> PSUM -> SBUF -> DRAM
    (+b3 and triangular mask on host)

Work tiling (uniform across 8 cores; per-core data selects tokens):
  256 i-slots per core: 8 width classes w_k = 1024-128k (k=0..7), 32 slots
  each (16 rows from batch-0 block k + 16 from batch-1 block k; core c takes
  rows 128k+16c..+16).  Window for class k = tokens [128k, 1024) of the
  owning batch.  Total pairs/core = 147456 (25% less than the 1024+512
  baseline units).  Diagonal-block overhang is masked on host.

Engine split is static per (stage, class), tuned from NTFF measurements:
  DVE tensor_scalar: H at 2x (bf16 SBUF), PSUM reads at 2x when PSUM tiles
  are bf16 (matmul writes bf16 PSUM), else 1x.  ACT activation: 1x + 293ns
  per instr -> gets the largest-FD chunks.  GPSIMD measured ~9Ge/s and
  starves the DVE SBUF port -> unused.
"""

import os
import numpy as np
import ml_dtypes

B, N, C = 2, 1024, 128
H1, H2 = 128, 64
NCORES = 8
BF16 = ml_dtypes.bfloat16

WIDTHS = [1024 - 128 * k for k in range(8)]
# flat y layout: class k block = 4 groups x [4v, 2e, w]
CLASS_OFF = np.cumsum([0] + [32 * w for w in WIDTHS])  # in elements

LAST_PERF = {}

# ---- engine split config (env-tunable): sets of classes handled by ACT ----
def _cfg(name, default):
    v = os.environ.get(name)
    if v is None:
        return set(default)
    return set(int(x) for x in v.split(",") if x != "")


def _split_sync_waits(bir_json, limit=1):
    """Walrus here accepts at most one sync-wait per instruction; move extra
    Tile-generated waits onto single-wait EventSemaphore carriers."""
    import json

    data = json.loads(bir_json)
    for f in data.get("functions", []):
        for blk in f.get("blocks", []):
            out = []
            for ins in blk.get("instructions", []):
                si = ins.get("sync_info")
                ow = (si or {}).get("on_wait") or []
                if len(ow) > limit:
                    for k, w in enumerate(ow[:-limit]):
                        out.append({
                            "debug": ins.get("debug", 0),
                            "engine": ins["engine"],
                            "name": f"{ins['name']}-xw{k}",
                            "opcode": "EventSemaphore",
                            "sync_info": {"on_update": [], "on_wait": [w]},
                        })
                    si["on_wait"] = ow[-limit:]
                out.append(ins)
            blk["instructions"] = out
    return json.dumps(data).encode()


def _install_compile_patch():
    import subprocess
    import concourse.bass_utils as bu
    import concourse.bass2jax as b2j

    if getattr(bu, "_fc_split_waits_patch", False):
        return
    orig = bu.compile_bir_kernel

    def patched(bir_json, tmpdir, neff_name="file.neff"):
        return orig(_split_sync_waits(bir_json), tmpdir, neff_name)

    bu._fc_split_waits_patch = True
    bu.compile_bir_kernel = patched
    b2j.compile_bir_kernel = patched

    if bool(int(os.environ.get("FC_LDW_OPT", "0"))):
        orig_cc = subprocess.check_call

        def cc(argv, *a, **kw):
            if (isinstance(argv, list) and argv
                    and "walrus_driver" in str(argv[0])):
                argv = ["--enable-ldw-opt=true"
                        if x == "--enable-ldw-opt=false" else x for x in argv]
            return orig_cc(argv, *a, **kw)

        subprocess.check_call = cc


def _build_program():
    import concourse.bass as bass
    import concourse.mybir as mybir
    from concourse.tile import TileContext

    f32 = mybir.dt.float32
    f32r = mybir.dt.float32r
    bf16 = mybir.dt.bfloat16

    psum_bf16 = bool(int(os.environ.get("FC_PSUM_BF16", "0")))
    ph_dt = bf16 if psum_bf16 else f32
    # classes whose H construct runs on ACT (rest on DVE)
    act_h = _cfg("FC_ACT_H", [])
    # classes whose h2s relu runs on ACT (rest on DVE)
    act_h2 = _cfg("FC_ACT_H2", [0, 1, 2, 3, 4, 5, 6, 7])
    # classes whose y copy runs on ACT (rest on DVE)
    act_y = _cfg("FC_ACT_Y", [2, 3, 4, 5, 6, 7])

    nc = bass.Bass()
    xr_d = nc.declare_dram_parameter("xr", [C, 256], f32r, isOutput=False)
    xw_d = nc.declare_dram_parameter("xw", [C, 2048], f32r, isOutput=False)
    w1l_d = nc.declare_dram_parameter("w1l", [C, H1], f32r, isOutput=False)
    w1r_d = nc.declare_dram_parameter("w1r", [C, H1], f32r, isOutput=False)
    b1c_d = nc.declare_dram_parameter("b1c", [H1, 1], f32, isOutput=False)
    w2b_d = nc.declare_dram_parameter("w2b", [H1, H2], bf16, isOutput=False)
    b2s_d = nc.declare_dram_parameter("b2s", [128, 1], f32, isOutput=False)
    w3s_d = nc.declare_dram_parameter("w3s", [128, 32], bf16, isOutput=False)
    y_d = nc.declare_dram_parameter("y", [1, 147456], bf16, isOutput=True)

    Relu = mybir.ActivationFunctionType.Relu
    Identity = mybir.ActivationFunctionType.Identity
    ADD = mybir.AluOpType.add
    MAX = mybir.AluOpType.max

    with TileContext(nc) as tc:
        with tc.tile_pool(name="const", bufs=1) as const:
            w1l_t = const.tile([C, H1], f32r, tag="w1l")
            w1r_t = const.tile([C, H1], f32r, tag="w1r")
            b1c_t = const.tile([H1, 1], f32, tag="b1c")
            w2b_t = const.tile([H1, H2], bf16, tag="w2b")
            b2s_t = const.tile([128, 1], f32, tag="b2s")
            w3s_t = const.tile([128, 32], bf16, tag="w3s")
            xr_t = const.tile([C, 256], f32r, tag="xr")
            xw_t = const.tile([C, 2048], f32r, tag="xw")
            aTb1_t = const.tile([H1, 256], f32, tag="aTb1")
            rT_t = const.tile([H1, 2048], bf16, tag="rT")

            nc.sync.dma_start(out=xw_t[:, 512:1024], in_=xw_d[:, 512:1024])
            nc.sync.dma_start(out=w1r_t, in_=w1r_d[:])
            nc.sync.dma_start(out=xr_t, in_=xr_d[:])
            nc.sync.dma_start(out=w1l_t, in_=w1l_d[:])
            for t, d in [(b1c_t, b1c_d), (w2b_t, w2b_d), (b2s_t, b2s_d),
                         (w3s_t, w3s_d)]:
                nc.sync.dma_start(out=t, in_=d[:])

            with tc.tile_pool(name="pre", bufs=2, space="PSUM") as pre:
                pa = pre.tile([128, 256], f32, tag="pa")
                nc.tensor.matmul(pa, lhsT=w1l_t, rhs=xr_t,
                                 start=True, stop=True)
                nc.vector.tensor_scalar(aTb1_t, pa, b1c_t, None, ADD)
                for ch in (1, 3, 0, 2):
                    if ch != 1:
                        nc.sync.dma_start(
                            out=xw_t[:, ch * 512:(ch + 1) * 512],
                            in_=xw_d[:, ch * 512:(ch + 1) * 512])
                    pr = pre.tile([128, 512], f32, tag="pr")
                    nc.tensor.matmul(pr, lhsT=w1r_t,
                                     rhs=xw_t[:, ch * 512:(ch + 1) * 512],
                                     start=True, stop=True)
                    nc.scalar.copy(rT_t[:, ch * 512:(ch + 1) * 512], pr)

            with (
                tc.tile_pool(name="Hp", bufs=8) as Hp,
                tc.tile_pool(name="h2p", bufs=8) as h2p,
                tc.tile_pool(name="yp", bufs=2) as yp,
                tc.tile_pool(name="php", bufs=3, space="PSUM") as php,
                tc.tile_pool(name="pyp", bufs=1, space="PSUM") as pyp,
            ):
                def _flush(pend):
                    (fw, fnq, fdm, fgrp, fh2ss, fysb, fact, fdma) = pend
                    py = pyp.tile([128, fw], ph_dt, tag="py")
                    for p4 in range(4):
                        v = p4
                        h2m = fh2ss[p4 // fdm]
                        hoff = (p4 % fdm) * fw
                        for q in range(fnq):
                            lo = hoff + 512 * q
                            hi = hoff + min(512 * (q + 1), fw)
                            nc.tensor.matmul(
                                py[32 * v:32 * v + 32,
                                   512 * q:512 * q + (hi - lo)],
                                lhsT=w3s_t, rhs=h2m[:, lo:hi],
                                start=True, stop=True,
                                tile_position=(0, 32 * v))
                    dst = fysb[:, fgrp * fw:(fgrp + 1) * fw]
                    if fact:
                        nc.scalar.activation(dst, py, Identity, bias=0.0)
                    else:
                        nc.vector.tensor_copy(dst, py)
                    if fdma is not None:
                        for v in range(4):
                            nc.sync.dma_start(
                                out=fdma[v],
                                in_=fysb[32 * v:32 * v + 2, :].rearrange(
                                    "e (g f) -> e g f", g=4))

                pending = None
                for k in [4, 5, 6, 7, 0, 1, 2, 3]:
                    w = WIDTHS[k]
                    nq = (w + 511) // 512 if not psum_bf16 else 1
                    # class block layout: [v=4, e=2, g=4, f=w]
                    yv = y_d[0, CLASS_OFF[k]:CLASS_OFF[k + 1]].rearrange(
                        "(v e g f) -> v e g f", v=4, e=2, g=4)
                    ysb = yp.tile([128, 4 * w], bf16, tag="ysb")
                    dm = 4 if k >= 6 else (2 if k >= 4 else 1)  # pairs merged per tile
                    for grp in range(4):
                        h2ss = []
                        for d0 in range(0, 4, dm):
                            Ht0 = Hp.tile([128, dm * w], bf16, tag="H0")
                            Ht1 = Hp.tile([128, dm * w], bf16, tag="H1")
                            Hts = [Ht0, Ht1]
                            for half in range(dm):
                                p = grp * 4 + d0 + half
                                b, rp = divmod(p, 8)
                                c0 = 32 * k + 16 * b + 2 * rp
                                off = 1024 * b + 128 * k
                                for e in range(2):
                                    hd = Hts[e][:, half * w:(half + 1) * w]
                                    if k in act_h:
                                        nc.scalar.activation(
                                            hd, rT_t[:, off:off + w], Relu,
                                            bias=aTb1_t[:, c0 + e:c0 + e + 1])
                                    else:
                                        nc.vector.tensor_scalar(
                                            hd, rT_t[:, off:off + w],
                                            aTb1_t[:, c0 + e:c0 + e + 1],
                                            0.0, ADD, op1=MAX)
                            mw = dm * w
                            nqm = (mw + 511) // 512
                            ph = php.tile([128, mw], ph_dt, tag="ph")
                            for e in range(2):
                                for q in range(nqm):
                                    qs = slice(512 * q, min(512 * (q + 1), mw))
                                    nc.tensor.matmul(
                                        ph[64 * e:64 * (e + 1), qs],
                                        lhsT=w2b_t, rhs=Hts[e][:, qs],
                                        start=True, stop=True,
                                        tile_position=(0, 64 * e))
                            h2s = h2p.tile([128, mw], bf16, tag="h2s")
                            if k in act_h2:
                                nc.scalar.activation(h2s, ph, Relu,
                                                     bias=b2s_t)
                            else:
                                nc.vector.tensor_scalar(h2s, ph, b2s_t, 0.0,
                                                        ADD, op1=MAX)
                            h2ss.append(h2s)
                        if pending is not None:
                            _flush(pending)
                        dma = yv if grp == 3 else None
                        pending = (w, nq, dm, grp, h2ss, ysb, k in act_y, dma)
                if pending is not None:
                    _flush(pending)
    return nc


def _pack_inputs(x, W1, b1, W2, b2, W3, b3):
    xT = np.ascontiguousarray(x.transpose(0, 2, 1)).astype(np.float32)
    w1l = np.ascontiguousarray(W1[:C]).astype(np.float32)
    w1r = np.ascontiguousarray(W1[C:]).astype(np.float32)
    b1c = np.ascontiguousarray(b1.reshape(H1, 1)).astype(np.float32)
    w2b = np.ascontiguousarray(W2).astype(BF16)
    b2s = np.concatenate([b2, b2]).reshape(128, 1).astype(np.float32)
    w3s = np.zeros((128, 32), dtype=BF16)
    w3s[0:64, 0] = W3[:, 0].astype(BF16)
    w3s[64:128, 1] = W3[:, 0].astype(BF16)
    xw = np.ascontiguousarray(np.concatenate([xT[0], xT[1]], axis=1))

    in_maps = []
    for c in range(NCORES):
        xr = np.empty((C, 256), dtype=np.float32)
        for k in range(8):
            for b in range(2):
                base = 128 * k + 16 * c
                xr[:, 32 * k + 16 * b:32 * k + 16 * b + 16] = \
                    xT[b][:, base:base + 16]
        in_maps.append({
            "xr": np.ascontiguousarray(xr), "xw": xw,
            "w1l": w1l, "w1r": w1r, "b1c": b1c, "w2b": w2b, "b2s": b2s,
            "w3s": w3s,
        })
    return in_maps


_TRIU = None


def _assemble(results, b3):
    global _TRIU
    y = np.zeros((B, N, N), dtype=np.float32)
    for c in range(NCORES):
        flat = results[c]["y"].reshape(-1).astype(np.float32)
        for k in range(8):
            w = WIDTHS[k]
            blk = flat[CLASS_OFF[k]:CLASS_OFF[k + 1]].reshape(4, 2, 4, w)
            for grp in range(4):
                for p4 in range(4):
                    p = grp * 4 + p4
                    b, rp = divmod(p, 8)
                    for e in range(2):
                        i = 128 * k + 16 * c + 2 * rp + e
                        y[b, i, 128 * k:128 * k + w] = blk[p4, e, grp]
    y += np.float32(b3[0])
    if _TRIU is None:
        _TRIU = np.triu(np.ones((N, N), dtype=np.float32), k=1)
    y *= _TRIU
    return y


def kernel(x, W1, b1, W2, b2, W3, b3):
    _install_compile_patch()
    from concourse.bass_utils import run_bass_kernel_spmd

    trace = bool(int(os.environ.get("FC_TRACE", "0")))
    nc = _build_program()
    in_maps = _pack_inputs(np.asarray(x), np.asarray(W1), np.asarray(b1),
                           np.asarray(W2), np.asarray(b2), np.asarray(W3),
                           np.asarray(b3))
    res = run_bass_kernel_spmd(nc, in_maps, core_ids=list(range(NCORES)),
                               trace=trace)
    LAST_PERF.clear()
    LAST_PERF.update({
        "exec_time_ns": res.exec_time_ns,
        "mean_exec_time_ns": res.mean_exec_time_ns,
        "trace": res.instructions_and_trace[1] if res.instructions_and_trace else None,
    })
    return _assemble(res.results, np.asarray(b3))
